# revision 1
# baseline (speedup 1.0000x reference)
"""Trainium2 Bass kernel for nn_RadialModel (forward NUFFT, radial MRI).

Per-core (1 frame, all 8 coils):
  1. coil multiply (DVE, bf16 out)       cimage = (xr+ixi)*(cr+ici)
  2. DFT via PE bf16 matmuls (two stages): G[v,u] = A @ (M^T @ A^T) with
     apodization + fftshift phases folded into the constant A matrices
  3. store grid to a DRAM table (bf16), coil-interleaved cells
     [p=v_pad(517), q=u_pad(517), cri(16)] with 2/3-cell wraparound halo
  4. Kaiser-Bessel interpolation with round-to-nearest centering so the
     fractional offset is in [-0.5, 0.5]: a 5x5 tap window then captures
     every tap with weight > 2.6e-3 (the 6th tap of the reference's 6x6
     always has |U| >= 2.5 there).  The table packs FOUR v-rows per cell
     (quad-row cells, 64 el), so one gather index fetches 5 q-cells x 4
     rows x 16 cri = 640B and a point needs only TWO quad-row taps (the
     8-row window always covers the 5 needed rows; stray rows get ~4e-6
     weights from the clamped KB polynomial).  One index per partition
     per call is a HW limit of the SWDGE indirect path (~1.25us fixed
     cost each) -> 256 calls, the dominant cost of the kernel.
  5. per-r4 weighted multiply + 40-tap reduce on DVE, sqrt(w) scale + store

Sharding: one frame (nt) per NeuronCore, 8 cores. Host does only
shard/reshape/unshuffle; all math on device.
"""
import math
import numpy as np

import concourse.bass as bass
import concourse.bacc as bacc
import concourse.mybir as mybir
import concourse.tile as tile
from concourse.bass_utils import run_bass_kernel_spmd
from concourse.masks import make_identity

F32 = mybir.dt.float32
I32 = mybir.dt.int32
AX = mybir.AxisListType
OP = mybir.AluOpType

IM = 256
G = 512
J = 6
JT = 5             # live taps per dim (rne centering => the dropped 6th
                   # tap always has |U| >= 2.5, KB weight <= 2.6e-3)
ALPHA = 2.34 * J
TWO_PI = 2.0 * np.pi
PAD = 517          # 512 + 2 left halo + 3 right halo
NT, NC, K = 8, 8, 16384
CELL = NC * 2      # floats per (p,q) cell = 16 (all coils interleaved)
TW = PAD * CELL    # table row width in elements = 8272
NTILE = 16         # point tiles of 1024 points (8 groups x 128 partitions)
GRP = 8            # groups per tile
DEG = 8            # KB weight polynomial degree (in t); abs err ~8.5e-6
NCELLS = PAD * PAD # flat cell count of the table


# ---------------------------------------------------------------- host consts
def _host_consts():
    # apodization correction 1/FT(kb)
    f = (np.arange(IM) - IM // 2) / G
    z = (np.pi * J * f) ** 2 - ALPHA ** 2
    s = np.sqrt(np.abs(z))
    val = np.where(z < 0, np.sinh(s) / np.maximum(s, 1e-12), np.sinc(s / np.pi))
    ftkb = (J / np.i0(ALPHA)) * val
    scal = 1.0 / ftkb
    # A[u, x'] = e^{i pi u/2 - 2 pi i u x'/G} * scal[x'] / sqrt(G)
    u = np.arange(G)[:, None].astype(np.float64)
    xp = np.arange(IM)[None, :].astype(np.float64)
    A = np.exp(1j * np.pi * u / 2 - 2j * np.pi * u * xp / G) * scal[None, :] / np.sqrt(G)
    art = np.ascontiguousarray(A.T.real, dtype=np.float32)   # [256, 512]
    ait = np.ascontiguousarray(A.T.imag, dtype=np.float32)
    aitn = np.ascontiguousarray(-A.T.imag, dtype=np.float32)
    # polynomial fit of w(t) = i0(ALPHA*sqrt(t))/i0(ALPHA) on t in [0,1]
    n = 512
    x = (1 - np.cos(np.pi * (np.arange(n) + 0.5) / n)) / 2
    w = np.i0(ALPHA * np.sqrt(x)) / np.i0(ALPHA)
    V = np.vander(x, DEG + 1, increasing=True)
    c, *_ = np.linalg.lstsq(V, w, rcond=None)
    return art, ait, aitn, c.astype(np.float64)


_ART, _AIT, _AITN, _CHEB = _host_consts()


# ---------------------------------------------------------------- bass build
def build_bass(debug=False):
    nc = bacc.Bacc()

    x_in = nc.declare_dram_parameter("x", [2, IM, IM], F32, isOutput=False)
    k_in = nc.declare_dram_parameter("kk", [2, K], F32, isOutput=False)
    c_in = nc.declare_dram_parameter("coil", [NC, 2, IM, IM], F32, isOutput=False)
    w_in = nc.declare_dram_parameter("wr", [128, NTILE * 128], F32, isOutput=False)
    art_in = nc.declare_dram_parameter("art", [IM, G], F32, isOutput=False)
    ait_in = nc.declare_dram_parameter("ait", [IM, G], F32, isOutput=False)
    aitn_in = nc.declare_dram_parameter("aitn", [IM, G], F32, isOutput=False)
    y_out = nc.declare_dram_parameter("yr", [128, NTILE * 128], F32, isOutput=True)

    BF16 = mybir.dt.bfloat16
    W2 = 4 * CELL          # quad-row cell: 4 v-rows x 16 cri = 64 el
    TW2 = PAD * W2         # table Q-row width = 33088 el
    PROWS = 130            # quads of table rows 0..519 (R = m + 4)
    T_dram = nc.dram_tensor("T0", [PROWS, TW2], BF16)

    CH = _CHEB
    with tile.TileContext(nc) as tc:
        with (
            tc.tile_pool(name="const", bufs=1) as constp,
            tc.tile_pool(name="work", bufs=1) as workp,
            tc.tile_pool(name="ctile", bufs=2) as coilp,
            tc.tile_pool(name="mtile", bufs=4) as mp,
            tc.tile_pool(name="bt", bufs=8) as btp,
            tc.tile_pool(name="stg", bufs=1) as stgp,
            tc.tile_pool(name="patch", bufs=2) as patchp,
            tc.tile_pool(name="w36", bufs=2) as w36p,
            tc.tile_pool(name="wp", bufs=2) as wpp,
            tc.tile_pool(name="rt", bufs=3) as resp,
            tc.tile_pool(name="ps1", bufs=4, space="PSUM") as ps1,
            tc.tile_pool(name="ps2", bufs=4, space="PSUM") as ps2,
        ):
            # ---------------- constants ----------------
            ident = constp.tile([128, 128], F32, tag="ident")
            make_identity(nc, ident[:])
            # A matrices: DMA f32, convert once to bf16 for PE
            art = []
            for name, src in (("art", art_in), ("ait", ait_in), ("aitn", aitn_in)):
                ts_ = []
                for xt in range(2):
                    tf = constp.tile([128, G], F32, tag=f"{name}f{xt}")
                    nc.sync.dma_start(out=tf[:], in_=src[xt * 128:(xt + 1) * 128, :])
                    tb = constp.tile([128, G], BF16, tag=f"{name}b{xt}")
                    nc.scalar.copy(out=tb[:], in_=tf[:])
                    # stage-2 copy with v-columns regrouped (r4, p) so the
                    # stride-4 quad slices become contiguous weight loads
                    tp_ = constp.tile([128, G], BF16, tag=f"{name}p{xt}")
                    pin = bass.AP(
                        tf[:].tensor, tf[:].offset,
                        [tf[:].ap[0], [1, 4], [4, 128]],
                    )
                    nc.scalar.copy(out=tp_[:], in_=pin)
                    ts_.append((tb, tp_))
                art.append(ts_)
            artT = [a[0] for a in art[0]]
            aitT = [a[0] for a in art[1]]
            aitnT = [a[0] for a in art[2]]
            artP = [a[1] for a in art[0]]
            aitP = [a[1] for a in art[1]]
            aitnP = [a[1] for a in art[2]]

            offs = constp.tile([128, JT], F32, tag="offs")
            for a in range(JT):
                nc.vector.memset(offs[:, a:a + 1], float(3 - (a + 1)))
            ylat = constp.tile([128, 8], F32, tag="ylat")
            for a in range(8):
                nc.vector.memset(ylat[:, a:a + 1], float(-a))

            # ---------------- k -> [p, c] transpose ----------------
            kg = workp.tile([128, 256], F32, tag="kg")  # [p, (d, c)]
            for d in range(2):
                kt_in = workp.tile([128, 128], F32, tag="ktin")
                nc.sync.dma_start(
                    out=kt_in[:], in_=k_in[d].rearrange("(c p) -> c p", p=128)
                )
                ktp = ps2.tile([128, 128], F32, tag="psb")
                nc.tensor.transpose(ktp[:], kt_in[:], ident[:])
                nc.scalar.copy(out=kg[:, d * 128:(d + 1) * 128], in_=ktp[:])

            # ---------------- w load + sqrt ----------------
            wsq = workp.tile([128, NTILE * 128], F32, tag="wsq")
            nc.sync.dma_start(out=wsq[:], in_=w_in[:])
            nc.scalar.activation(
                out=wsq[:], in_=wsq[:],
                func=mybir.ActivationFunctionType.Sqrt,
            )

            # ---------------- index & weight math (DVE) ----------------
            # gx = om*(G/2pi); gx += 512 if gx < 0  -> [0, 512)
            gx0 = workp.tile([128, 256], F32, tag="gx0")
            nc.vector.tensor_scalar_mul(gx0[:], kg[:], float(G / TWO_PI))
            msk = workp.tile([128, 256], F32, tag="msk")
            nc.vector.tensor_scalar(
                out=msk[:], in0=gx0[:], scalar1=0.0, scalar2=None, op0=OP.is_lt
            )
            gxy = workp.tile([128, 256], F32, tag="gxy")
            nc.vector.scalar_tensor_tensor(
                out=gxy[:], in0=msk[:], scalar=float(G), in1=gx0[:],
                op0=OP.mult, op1=OP.add,
            )
            # gm3 = gxy - 3 ; f = rne(gm3 - 0.498) via 2^23 trick ; r = gm3 - f
            gm3 = workp.tile([128, 256], F32, tag="gm3")
            nc.vector.tensor_scalar(
                out=gm3[:], in0=gxy[:], scalar1=3.0, scalar2=None, op0=OP.subtract
            )
            fl = workp.tile([128, 256], F32, tag="fl")
            nc.vector.tensor_scalar(
                out=fl[:], in0=gm3[:],
                scalar1=0.0, scalar2=12582912.0,
                op0=OP.add, op1=OP.add,
            )
            nc.vector.tensor_scalar(
                out=fl[:], in0=fl[:], scalar1=12582912.0, scalar2=None,
                op0=OP.subtract,
            )
            rr = workp.tile([128, 256], F32, tag="rr")
            nc.vector.tensor_sub(rr[:], gm3[:], fl[:])

            # Q0 = floor((fl_y + 5)/4): h = fl*0.25 + 1.25, rne(h - 0.498)
            fp = workp.tile([128, 128], F32, tag="fp")
            nc.vector.tensor_scalar(
                out=fp[:], in0=fl[:, 128:256], scalar1=0.25, scalar2=1.25,
                op0=OP.mult, op1=OP.add,
            )
            nc.vector.tensor_scalar(
                out=fp[:], in0=fp[:],
                scalar1=-0.498046875, scalar2=12582912.0,
                op0=OP.add, op1=OP.add,
            )
            nc.vector.tensor_scalar(
                out=fp[:], in0=fp[:], scalar1=12582912.0, scalar2=None,
                op0=OP.subtract,
            )
            # sY = gy + 4 - 4*Q0 = gm3_y + 7 - 4*fp; y-tap j weight
            # arg = sY - j for gathered rows 4*Q0 + j, j = 0..7 (R = m + 4)
            sY = workp.tile([128, 128], F32, tag="sY")
            nc.vector.scalar_tensor_tensor(
                out=sY[:], in0=fp[:], scalar=-4.0, in1=gm3[:, 128:256],
                op0=OP.mult, op1=OP.add,
            )
            nc.vector.tensor_scalar_add(sY[:], sY[:], 7.0)

            # tap weight args: x: rr_x + offs (5); y: sY - j (6)
            NXC = 128 * JT
            ut = workp.tile([128, NXC + 128 * 8], F32, tag="ut")
            utx3 = ut[:, 0:NXC].rearrange("p (c a) -> p c a", a=JT)
            nc.vector.tensor_tensor(
                out=utx3,
                in0=rr[:, 0:128].unsqueeze(2).broadcast_to([128, 128, JT]),
                in1=offs[:].unsqueeze(1).broadcast_to([128, 128, JT]),
                op=OP.add,
            )
            uty3 = ut[:, NXC:].rearrange("p (c j) -> p c j", j=8)
            nc.vector.tensor_tensor(
                out=uty3,
                in0=sY[:].unsqueeze(2).broadcast_to([128, 128, 8]),
                in1=ylat[:].unsqueeze(1).broadcast_to([128, 128, 8]),
                op=OP.add,
            )
            # t = max(0, 1 - (U/3)^2)
            tsq = workp.tile([128, 128 * JT + 128 * 8], F32, tag="tsq")
            nc.vector.tensor_mul(tsq[:], ut[:], ut[:])
            nc.vector.tensor_scalar(
                out=tsq[:], in0=tsq[:], scalar1=float(-1.0 / 9.0), scalar2=1.0,
                op0=OP.mult, op1=OP.add,
            )
            nc.vector.tensor_scalar_max(tsq[:], tsq[:], 0.0)
            # Horner in t
            acc = workp.tile([128, 128 * JT + 128 * 8], F32, tag="acc")
            nc.vector.tensor_scalar(
                out=acc[:], in0=tsq[:], scalar1=float(CH[DEG]),
                scalar2=float(CH[DEG - 1]), op0=OP.mult, op1=OP.add,
            )
            for d in range(DEG - 2, -1, -1):
                nc.vector.tensor_mul(acc[:], acc[:], tsq[:])
                nc.vector.tensor_scalar_add(acc[:], acc[:], float(CH[d]))
            # acc = W_all [p, (d, c, a)]: d=0 -> wx taps, d=1 -> wy taps

            # gather cell4 indices: (Q0 + b2)*517 + 3 + fx, b2 = 0..1
            cbt = constp.tile([128, 2], F32, tag="cbt")
            for a in range(2):
                nc.vector.memset(cbt[:, a:a + 1], float(a * PAD + 3))
            fy517 = workp.tile([128, 128], F32, tag="fy517")
            nc.vector.tensor_scalar_mul(fy517[:], fp[:], float(PAD))
            idxf = workp.tile([128, 128 * 2], F32, tag="idxf")
            idxf3 = idxf[:].rearrange("p (c b) -> p c b", b=2)
            nc.vector.tensor_tensor(
                out=idxf3,
                in0=fy517[:].unsqueeze(2).broadcast_to([128, 128, 2]),
                in1=cbt[:].unsqueeze(1).broadcast_to([128, 128, 2]),
                op=OP.add,
            )
            nc.vector.tensor_tensor(
                out=idxf3,
                in0=idxf3,
                in1=fl[:, 0:128].unsqueeze(2).broadcast_to([128, 128, 2]),
                op=OP.add,
            )
            idx32 = workp.tile([128, 128 * 2], I32, tag="idx32")
            nc.vector.tensor_copy(out=idx32[:], in_=idxf[:])

            # x image tiles (persist across all coils)
            xts = []
            for xt in range(2):
                xt_t = workp.tile([128, 2 * IM], F32, tag=f"xt{xt}")
                nc.sync.dma_start(
                    out=xt_t[:],
                    in_=x_in[:, xt * 128:(xt + 1) * 128, :]
                    .rearrange("ri x y -> x ri y"),
                )
                xts.append(xt_t)

            # 1 persistent bf16 staging; partition p holds the v-row quad
            # (4p .. 4p+3), cells4 laid out (q, r4, e)
            stg0 = stgp.tile([128, G * W2], BF16, tag="stg0")

            for c in range(NC):
                # ---- coil multiply (bf16 out for PE) ----
                mt = []
                for xt in range(2):
                    ct = coilp.tile([128, 2 * IM], F32, tag="ct")
                    nc.sync.dma_start(
                        out=ct[:],
                        in_=c_in[c, :, xt * 128:(xt + 1) * 128, :]
                        .rearrange("ri x y -> x ri y"),
                    )
                    xt_t = xts[xt]
                    m = mp.tile([128, 2 * IM], BF16, tag="m")
                    xr, xi = xt_t[:, 0:IM], xt_t[:, IM:2 * IM]
                    cr, ci = ct[:, 0:IM], ct[:, IM:2 * IM]
                    mr, mi = m[:, 0:IM], m[:, IM:2 * IM]
                    t1 = mp.tile([128, IM], F32, tag="cm1")
                    t2 = mp.tile([128, IM], F32, tag="cm2")
                    nc.vector.tensor_mul(t1[:], xr, cr)
                    nc.vector.tensor_mul(t2[:], xi, ci)
                    nc.vector.tensor_sub(mr, t1[:], t2[:])
                    nc.vector.tensor_mul(t1[:], xr, ci)
                    nc.vector.tensor_mul(t2[:], xi, cr)
                    nc.vector.tensor_add(mi, t1[:], t2[:])
                    mt.append(m)
                # ---- stage 1: BT[y, u] per (ri, Yt) ----
                bt = {}
                for yt in range(2):
                    pr = ps1.tile([128, G], F32, tag="psa")
                    pi = ps1.tile([128, G], F32, tag="psa")
                    for xt in range(2):
                        mrb = mt[xt][:, yt * 128:yt * 128 + 128]
                        mib = mt[xt][:, IM + yt * 128:IM + yt * 128 + 128]
                        st = xt == 0
                        sp = xt == 1
                        nc.tensor.matmul(pr[:], mrb, artT[xt][:], start=st, stop=False)
                        nc.tensor.matmul(pi[:], mrb, aitT[xt][:], start=st, stop=False)
                        nc.tensor.matmul(pr[:], mib, aitnT[xt][:], start=False, stop=sp)
                        nc.tensor.matmul(pi[:], mib, artT[xt][:], start=False, stop=sp)
                    btr = btp.tile([128, G], BF16, tag="bt")
                    bti = btp.tile([128, G], BF16, tag="bt")
                    nc.scalar.copy(out=btr[:], in_=pr[:])
                    nc.scalar.copy(out=bti[:], in_=pi[:])
                    bt[(0, yt)] = btr
                    bt[(1, yt)] = bti
                # ---- stage 2: G[v, u] with v = 4p + r4 via stride-4 A
                # column slices; drain into quad-row staging ----
                for r2 in range(4):
                    stg3 = stg0[:].rearrange("p (q w) -> p q w", w=W2)
                    gr = ps2.tile([128, G], F32, tag="psb")
                    gi = ps2.tile([128, G], F32, tag="psb")
                    for yt in range(2):
                        av = artP[yt][:, r2 * 128:(r2 + 1) * 128]
                        aiv = aitP[yt][:, r2 * 128:(r2 + 1) * 128]
                        ainv = aitnP[yt][:, r2 * 128:(r2 + 1) * 128]
                        btr = bt[(0, yt)]
                        bti = bt[(1, yt)]
                        st = yt == 0
                        sp = yt == 1
                        nc.tensor.matmul(gr[:], av, btr[:], start=st, stop=False)
                        nc.tensor.matmul(gi[:], aiv, btr[:], start=st, stop=False)
                        nc.tensor.matmul(gr[:], ainv, bti[:], start=False, stop=sp)
                        nc.tensor.matmul(gi[:], av, bti[:], start=False, stop=sp)
                    c2 = r2 * CELL + 2 * c
                    # split strided drains across Scalar and Vector engines
                    if c % 2 == 0:
                        nc.scalar.copy(out=stg3[:, :, c2:c2 + 1], in_=gr[:].unsqueeze(2))
                        nc.vector.tensor_copy(out=stg3[:, :, c2 + 1:c2 + 2], in_=gi[:].unsqueeze(2))
                    else:
                        nc.vector.tensor_copy(out=stg3[:, :, c2:c2 + 1], in_=gr[:].unsqueeze(2))
                        nc.scalar.copy(out=stg3[:, :, c2 + 1:c2 + 2], in_=gi[:].unsqueeze(2))

            # ---- table stores: main (Q 1..128) + q halos, then halo
            # quads Q0 (v 508..511 <- stg[127]) and Q129 (v 0..3 <- stg[0])
            t_stores = []
            Th = T_dram
            t_stores.append(nc.sync.dma_start(
                out=Th[1:129, 2 * W2:2 * W2 + G * W2], in_=stg0[:]
            ))
            t_stores.append(nc.sync.dma_start(
                out=Th[1:129, 514 * W2:517 * W2], in_=stg0[:, 0:3 * W2],
            ))
            t_stores.append(nc.sync.dma_start(
                out=Th[1:129, 0:2 * W2], in_=stg0[:, 510 * W2:512 * W2],
            ))
            for dst, psrc in ((0, 127), (129, 0)):
                t_stores += [
                    nc.sync.dma_start(
                        out=Th[dst:dst + 1, 2 * W2:2 * W2 + G * W2],
                        in_=stg0[psrc:psrc + 1, :],
                    ),
                    nc.sync.dma_start(
                        out=Th[dst:dst + 1, 514 * W2:517 * W2],
                        in_=stg0[psrc:psrc + 1, 0:3 * W2],
                    ),
                    nc.sync.dma_start(
                        out=Th[dst:dst + 1, 0:2 * W2],
                        in_=stg0[psrc:psrc + 1, 510 * W2:512 * W2],
                    ),
                ]

            # ======== gather + combine ========
            # per index: 320 contiguous el (5 cells4 = 5q x 4rows x 16cri,
            # 640B); 2 quad-taps/point, 1 idx/partition/call -> 256 calls
            tab_flat = T_dram[:].rearrange("r (q e) -> (r q) e", e=W2)
            all_gathers = []
            for t in range(NTILE):
                # W[g, r4, b2, a] = wy[g, 4*b2 + r4] * wx[g, a]
                w240 = w36p.tile([128, GRP * 40], F32, tag="w36")
                for r2 in range(4):
                    ow = bass.AP(
                        w240[:].tensor, w240[:].offset + r2 * 10,
                        [w240[:].ap[0], [40, GRP], [5, 2], [1, 5]],
                    )
                    wyv = bass.AP(
                        acc[:].tensor,
                        acc[:].offset + 128 * JT + t * GRP * 8 + r2,
                        [acc[:].ap[0], [8, GRP], [4, 2], [0, 5]],
                    )
                    wxv = bass.AP(
                        acc[:].tensor, acc[:].offset + t * GRP * JT,
                        [acc[:].ap[0], [JT, GRP], [0, 2], [1, 5]],
                    )
                    nc.vector.tensor_tensor(out=ow, in0=wyv, in1=wxv, op=OP.mult)
                patch = patchp.tile([128, GRP * 2 * JT * W2], BF16, tag="patch")
                for g in range(GRP):
                    for b in range(2):
                        col = (t * GRP + g) * 2 + b
                        gi_ = nc.gpsimd.indirect_dma_start(
                            out=patch[:, (g * 2 + b) * JT * W2:
                                      (g * 2 + b + 1) * JT * W2],
                            out_offset=None,
                            in_=tab_flat,
                            in_offset=bass.IndirectOffsetOnAxis(
                                ap=idx32[:, col:col + 1], axis=0
                            ),
                        )
                        all_gathers.append(gi_)
                # WP[p, (g, cr, (r4, b2, a))] = patch[p, (g, b2, a, r4, cr)] * W
                wp = wpp.tile([128, GRP * 40 * CELL], BF16, tag="wpt")
                for r2 in range(4):
                    pv = bass.AP(
                        patch[:].tensor, patch[:].offset + r2 * CELL,
                        [patch[:].ap[0],
                         [2 * JT * W2, GRP], [1, CELL], [W2, 10]],
                    )
                    wv = bass.AP(
                        w240[:].tensor, w240[:].offset + r2 * 10,
                        [w240[:].ap[0], [40, GRP], [0, CELL], [1, 10]],
                    )
                    ov = bass.AP(
                        wp[:].tensor, wp[:].offset + r2 * 10,
                        [wp[:].ap[0],
                         [40 * CELL, GRP], [40, CELL], [1, 10]],
                    )
                    nc.vector.tensor_tensor(out=ov, in0=pv, in1=wv, op=OP.mult)
                # reduce innermost 40 -> private per-tile result tile
                # (slicing a shared accumulator would serialize the gather
                # pipeline on whole-tile WAR hazards)
                rt = resp.tile([128, 128], F32, tag="rt")
                rv = bass.AP(
                    rt[:].tensor, rt[:].offset,
                    [rt[:].ap[0], [16, GRP], [1, CELL]],
                )
                wp3 = wp[:].rearrange("p (g cr ba) -> p g cr ba", cr=CELL, ba=40)
                nc.vector.tensor_reduce(out=rv, in_=wp3, axis=AX.X, op=OP.add)
                ts_ = slice(t * 128, (t + 1) * 128)
                nc.vector.tensor_mul(rt[:], rt[:], wsq[:, ts_])
                nc.sync.dma_start(out=y_out[:, ts_], in_=rt[:])

            # explicit RAW edges: gathers after table stores
            for gi_ in all_gathers:
                for si in t_stores:
                    tile.add_dep_helper(gi_.ins, si.ins, reason="T RAW")



            if debug:
                dbg_outs = {
                    "kgo": kg, "acco": acc, "idxo": idx32, "flo": fl, "rro": rr,
                }
                for nm, t_ in dbg_outs.items():
                    o = nc.dram_tensor(nm, list(t_[:].shape), t_[:].dtype,
                                       kind="ExternalOutput")
                    nc.sync.dma_start(out=o[:], in_=t_[:])
                o = nc.dram_tensor("t0o", [PAD, TW], BF16, kind="ExternalOutput")
                di = nc.sync.dma_start(out=o[:], in_=T_dram[:])
                for si in t_stores:
                    tile.add_dep_helper(di.ins, si.ins, reason="T dump RAW")

    nc.compile()
    return nc


_NC_CACHE = None


def _get_nc():
    global _NC_CACHE
    if _NC_CACHE is None:
        _NC_CACHE = build_bass()
    return _NC_CACHE


# ---------------------------------------------------------------- host glue
def _shuffle_w(w_t):
    # w[c, ri, K] -> [p, (t, g, c, ri)] with K = t*1024 + g*128 + p
    v = w_t.reshape(NC, 2, NTILE, GRP, 128)
    return np.ascontiguousarray(v.transpose(4, 2, 3, 0, 1).reshape(128, NTILE * 128))


def _unshuffle_y(yr):
    # [p, (t, g, c, ri)] -> y[c, ri, K]
    v = yr.reshape(128, NTILE, GRP, NC, 2)
    return np.ascontiguousarray(v.transpose(3, 4, 1, 2, 0).reshape(NC, 2, K))


def make_in_maps(x, k, coil_sensitivities, w):
    in_maps = []
    coil0 = np.ascontiguousarray(coil_sensitivities[0], dtype=np.float32)
    for t in range(NT):
        in_maps.append({
            "x": np.ascontiguousarray(x[t], dtype=np.float32),
            "kk": np.ascontiguousarray(k[t], dtype=np.float32),
            "coil": coil0,
            "wr": _shuffle_w(np.asarray(w[t], dtype=np.float32)),
            "art": _ART, "ait": _AIT, "aitn": _AITN,
        })
    return in_maps


def run(x, k, coil_sensitivities, w, trace=False, **spmd_kwargs):
    nc = _get_nc()
    in_maps = make_in_maps(x, k, coil_sensitivities, w)
    r = run_bass_kernel_spmd(nc, in_maps, list(range(NT)), trace=trace, **spmd_kwargs)
    y = np.stack([_unshuffle_y(r.results[t]["yr"]) for t in range(NT)], axis=0)
    return y.astype(np.float32), r


def kernel(x, k, coil_sensitivities, w):
    y, _ = run(x, k, coil_sensitivities, w, trace=False)
    return y



# revision 11
# speedup vs baseline: 1.0361x; 1.0361x over previous
"""Trainium2 Bass kernel for nn_RadialModel (forward NUFFT, radial MRI).

Per-core (1 frame, all 8 coils):
  1. coil multiply (DVE, bf16 out)       cimage = (xr+ixi)*(cr+ici)
  2. DFT via PE bf16 matmuls (two stages): G[v,u] = A @ (M^T @ A^T) with
     apodization + fftshift phases folded into the constant A matrices
  3. store grid to TWO DRAM tables (parity-split over v-quads): table rows
     hold one v-quad (4 rows) of 516 q-cells, cell = [4 v-rows x 16 cri]
     bf16 (128B).  Row pitch 33024 el = 129 x 512B, so a (row, q-quad)
     address is idx*512B with idx = row*129 + u <= 8254 (int16-safe).
  4. bulk gather via gpsimd.dma_gather (one instruction per 1024 points
     per parity): per point fetch 8 q-cells (1KB) 4-aligned covering the
     5-tap x window; the two v-quads (Q0, Q0+1) have opposite parity so
     the even/odd tables each serve exactly one tap per point.
  5. combine on DVE: one broadcast multiply by the outer-product KB
     weights (w <= 2.6e-3 outside the true 5x5 window), then a pairwise
     add tree (bf16, final levels f32), sqrt(w) scale + store.

Index math runs twice: once in point-major layout [p=K%128, c] for the
weights, once in a fold layout [pd'*16+q, c'*8+pd] (fed by a
host-pre-shuffled copy of k) that makes the int16 idx tensor land in the
[i%16, i//16] wrapped layout dma_gather requires after a contiguous DRAM
round-trip with 256B-run replica reads.

Sharding: one frame (nt) per NeuronCore, 8 cores. Host does only
shard/reshape/unshuffle; all math on device.
"""
import math
import numpy as np

import concourse.bass as bass
import concourse.bacc as bacc
import concourse.mybir as mybir
import concourse.tile as tile
from concourse.bass_utils import run_bass_kernel_spmd
from concourse.masks import make_identity

F32 = mybir.dt.float32
I32 = mybir.dt.int32
I16 = mybir.dt.int16
BF16 = mybir.dt.bfloat16
AX = mybir.AxisListType
OP = mybir.AluOpType

IM = 256
G = 512
J = 6
ALPHA = 2.34 * J
TWO_PI = 2.0 * np.pi
PAD = 516          # 512 + 2 left halo + 2 right halo q-cells
NT, NC, K = 8, 8, 16384
CELL = NC * 2      # floats per (v,q) cell-column = 16 (all coils)
NTILE = 16         # point tiles of 1024 points
GRP = 8            # point groups (of 128) per tile
DEG = 8            # KB weight polynomial degree (in t)
W2 = 4 * CELL      # quad-row cell: 4 v-rows x 16 cri = 64 el (128B)
TROW = PAD * W2    # table row: 516 cells = 33024 el = 129 x 256 el units
RMAGIC = 12582912.0  # 1.5 * 2^23 (f32 rne-floor trick)
FEPS = -0.498046875


# ---------------------------------------------------------------- host consts
def _host_consts():
    # apodization correction 1/FT(kb)
    f = (np.arange(IM) - IM // 2) / G
    z = (np.pi * J * f) ** 2 - ALPHA ** 2
    s = np.sqrt(np.abs(z))
    val = np.where(z < 0, np.sinh(s) / np.maximum(s, 1e-12), np.sinc(s / np.pi))
    ftkb = (J / np.i0(ALPHA)) * val
    scal = 1.0 / ftkb
    # A[u, x'] = e^{i pi u/2 - 2 pi i u x'/G} * scal[x'] / sqrt(G)
    u = np.arange(G)[:, None].astype(np.float64)
    xp = np.arange(IM)[None, :].astype(np.float64)
    A = np.exp(1j * np.pi * u / 2 - 2j * np.pi * u * xp / G) * scal[None, :] / np.sqrt(G)
    art = np.ascontiguousarray(A.T.real, dtype=np.float32)   # [256, 512]
    ait = np.ascontiguousarray(A.T.imag, dtype=np.float32)
    aitn = np.ascontiguousarray(-A.T.imag, dtype=np.float32)
    # polynomial fit of w(t) = i0(ALPHA*sqrt(t))/i0(ALPHA) on t in [0,1]
    n = 512
    x = (1 - np.cos(np.pi * (np.arange(n) + 0.5) / n)) / 2
    w = np.i0(ALPHA * np.sqrt(x)) / np.i0(ALPHA)
    V = np.vander(x, DEG + 1, increasing=True)
    c, *_ = np.linalg.lstsq(V, w, rcond=None)
    return art, ait, aitn, c.astype(np.float64)


_ART, _AIT, _AITN, _CHEB = _host_consts()


# stage-2 A-column permutation: out partition p holds grid v-quad V(p) so
# that stg0 partitions 0..63 are exactly even-table rows 0..63 (quad R=2p)
# and partitions 64..127 the odd-table rows (R=2(p-64)+1).  Table quad R
# covers padded rows 4R..4R+3 = grid rows 4R-4..4R-1 (mod 512) = grid
# v-quad V = (R-1) mod 128.
def _vquad(p):
    r = 2 * p if p < 64 else 2 * (p - 64) + 1
    return (r - 1) % 128


# ---------------------------------------------------------------- bass build
def build_bass(debug=False):
    nc = bacc.Bacc()

    x_in = nc.declare_dram_parameter("x", [2, IM, IM], F32, isOutput=False)
    k_in = nc.declare_dram_parameter("kk", [2, K], F32, isOutput=False)
    kf_in = nc.declare_dram_parameter("kf", [2, 128, 128], F32, isOutput=False)
    c_in = nc.declare_dram_parameter("coil", [NC, 2, IM, IM], F32, isOutput=False)
    w_in = nc.declare_dram_parameter("wr", [128, NTILE * 128], F32, isOutput=False)
    art_in = nc.declare_dram_parameter("art", [IM, G], F32, isOutput=False)
    ait_in = nc.declare_dram_parameter("ait", [IM, G], F32, isOutput=False)
    aitn_in = nc.declare_dram_parameter("aitn", [IM, G], F32, isOutput=False)
    y_out = nc.declare_dram_parameter("yr", [128, NTILE * 128], F32, isOutput=True)

    TE = nc.dram_tensor("TE", [64, TROW], BF16)     # even v-quads (R=2e)
    TO = nc.dram_tensor("TO", [64, TROW], BF16)     # odd v-quads (R=2o+1)
    D4 = nc.dram_tensor("D4", [128, 256], I16)      # idx fold round-trip

    CH = _CHEB
    with tile.TileContext(nc) as tc:
        with (
            tc.tile_pool(name="const", bufs=1) as constp,
            tc.tile_pool(name="work", bufs=1) as workp,
            tc.tile_pool(name="ctile", bufs=2) as coilp,
            tc.tile_pool(name="mtile", bufs=4) as mp,
            tc.tile_pool(name="bt", bufs=8) as btp,
            tc.tile_pool(name="stg", bufs=1) as stgp,
            tc.tile_pool(name="patch", bufs=3) as patchp,
            tc.tile_pool(name="rt", bufs=3) as resp,
            tc.tile_pool(name="ps1", bufs=4, space="PSUM") as ps1,
            tc.tile_pool(name="ps2", bufs=4, space="PSUM") as ps2,
        ):
            # ---------------- constants ----------------
            ident = constp.tile([128, 128], F32, tag="ident")
            make_identity(nc, ident[:])
            # A matrices: bf16 via cast-during-DMA (SWDGE)
            artT, aitT, aitnT = [], [], []
            artP, aitP, aitnP = [], [], []
            for name, src, dT, dP in (
                ("art", art_in, artT, artP),
                ("ait", ait_in, aitT, aitP),
                ("aitn", aitn_in, aitnT, aitnP),
            ):
                for xt in range(2):
                    tb = constp.tile([128, G], BF16, tag=f"{name}b{xt}")
                    nc.gpsimd.dma_start(
                        out=tb[:], in_=src[xt * 128:(xt + 1) * 128, :]
                    )
                    # stage-2 copy with v-columns regrouped so slice r2,
                    # col j reads A column 4*V(j)+r2 (V = _vquad)
                    tp_ = constp.tile([128, G], BF16, tag=f"{name}p{xt}")
                    tbap = tb[:]
                    # j = 0: A col 508 + r2
                    nc.scalar.copy(
                        out=bass.AP(tp_[:].tensor, tp_[:].offset,
                                    [tp_[:].ap[0], [128, 4]]),
                        in_=bass.AP(tbap.tensor, tbap.offset + 508,
                                    [tbap.ap[0], [1, 4]]),
                    )
                    # j = 1..63: A col 8j - 4 + r2
                    nc.scalar.copy(
                        out=bass.AP(tp_[:].tensor, tp_[:].offset + 1,
                                    [tp_[:].ap[0], [128, 4], [1, 63]]),
                        in_=bass.AP(tbap.tensor, tbap.offset + 4,
                                    [tbap.ap[0], [1, 4], [8, 63]]),
                    )
                    # j = 64..127: A col 8j - 512 + r2
                    nc.scalar.copy(
                        out=bass.AP(tp_[:].tensor, tp_[:].offset + 64,
                                    [tp_[:].ap[0], [128, 4], [1, 64]]),
                        in_=bass.AP(tbap.tensor, tbap.offset,
                                    [tbap.ap[0], [1, 4], [8, 64]]),
                    )
                    dT.append(tb)
                    dP.append(tp_)

            offs8 = constp.tile([128, 8], F32, tag="offs8")
            for a in range(8):
                nc.vector.memset(offs8[:, a:a + 1], float(-a))
            ylat4 = constp.tile([128, 4], F32, tag="ylat4")
            for a in range(4):
                nc.vector.memset(ylat4[:, a:a + 1], float(-a))

            # ---------------- k -> [p, c] transpose ----------------
            kg = workp.tile([128, 256], F32, tag="kg")  # [p, (d, c)]
            for d in range(2):
                kt_in = workp.tile([128, 128], F32, tag="ktin")
                nc.sync.dma_start(
                    out=kt_in[:], in_=k_in[d].rearrange("(c p) -> c p", p=128)
                )
                ktp = ps2.tile([128, 128], F32, tag="psb")
                nc.tensor.transpose(ktp[:], kt_in[:], ident[:])
                nc.scalar.copy(out=kg[:, d * 128:(d + 1) * 128], in_=ktp[:])

            # ---------------- w load + sqrt ----------------
            wsq = workp.tile([128, NTILE * 128], F32, tag="wsq")
            nc.sync.dma_start(out=wsq[:], in_=w_in[:])
            nc.scalar.activation(
                out=wsq[:], in_=wsq[:],
                func=mybir.ActivationFunctionType.Sqrt,
            )

            # ======== point-major index & weight math (DVE) ========
            # gx = om*(G/2pi); gx += 512 if gx < 0  -> [0, 512)
            gx0 = workp.tile([128, 256], F32, tag="gx0")
            nc.vector.tensor_scalar_mul(gx0[:], kg[:], float(G / TWO_PI))
            msk = workp.tile([128, 256], F32, tag="msk")
            nc.vector.tensor_scalar(
                out=msk[:], in0=gx0[:], scalar1=0.0, scalar2=None, op0=OP.is_lt
            )
            gxy = workp.tile([128, 256], F32, tag="gxy")
            nc.vector.scalar_tensor_tensor(
                out=gxy[:], in0=msk[:], scalar=float(G), in1=gx0[:],
                op0=OP.mult, op1=OP.add,
            )
            # gm3 = gxy - 3 ; fl = rne(gm3 - 0.498) ; rr = gm3 - fl
            gm3 = workp.tile([128, 256], F32, tag="gm3")
            nc.vector.tensor_scalar(
                out=gm3[:], in0=gxy[:], scalar1=3.0, scalar2=None, op0=OP.subtract
            )
            fl = workp.tile([128, 256], F32, tag="fl")
            nc.vector.tensor_scalar(
                out=fl[:], in0=gm3[:], scalar1=FEPS, scalar2=RMAGIC,
                op0=OP.add, op1=OP.add,
            )
            nc.vector.tensor_scalar(
                out=fl[:], in0=fl[:], scalar1=RMAGIC, scalar2=None,
                op0=OP.subtract,
            )
            rr = workp.tile([128, 256], F32, tag="rr")
            nc.vector.tensor_sub(rr[:], gm3[:], fl[:])

            # u = floor((fl_x+3)/4) ; x window cells 4u..4u+7
            ux = workp.tile([128, 128], F32, tag="ux")
            nc.vector.tensor_scalar(
                out=ux[:], in0=fl[:, 0:128], scalar1=0.25,
                scalar2=0.75 + FEPS, op0=OP.mult, op1=OP.add,
            )
            nc.vector.tensor_scalar_add(ux[:], ux[:], RMAGIC)
            nc.vector.tensor_scalar(
                out=ux[:], in0=ux[:], scalar1=RMAGIC, scalar2=None,
                op0=OP.subtract,
            )
            # x tap arg base: col 4u+a has arg (gm3_x + 5 - 4u) - a
            xbase = workp.tile([128, 128], F32, tag="xbase")
            nc.vector.scalar_tensor_tensor(
                out=xbase[:], in0=ux[:], scalar=-4.0, in1=gm3[:, 0:128],
                op0=OP.mult, op1=OP.add,
            )
            nc.vector.tensor_scalar_add(xbase[:], xbase[:], 5.0)

            # fp = Q0 = floor((fl_y+5)/4)
            fp = workp.tile([128, 128], F32, tag="fp")
            nc.vector.tensor_scalar(
                out=fp[:], in0=fl[:, 128:256], scalar1=0.25,
                scalar2=1.25 + FEPS, op0=OP.mult, op1=OP.add,
            )
            nc.vector.tensor_scalar_add(fp[:], fp[:], RMAGIC)
            nc.vector.tensor_scalar(
                out=fp[:], in0=fp[:], scalar1=RMAGIC, scalar2=None,
                op0=OP.subtract,
            )
            # sY = gy + 4 - 4*Q0 ; par = Q0 mod 2
            sY = workp.tile([128, 128], F32, tag="sY")
            nc.vector.scalar_tensor_tensor(
                out=sY[:], in0=fp[:], scalar=-4.0, in1=gm3[:, 128:256],
                op0=OP.mult, op1=OP.add,
            )
            nc.vector.tensor_scalar_add(sY[:], sY[:], 7.0)
            q2 = workp.tile([128, 128], F32, tag="q2")
            nc.vector.tensor_scalar(
                out=q2[:], in0=fp[:], scalar1=0.5, scalar2=-0.25,
                op0=OP.mult, op1=OP.add,
            )
            nc.vector.tensor_scalar_add(q2[:], q2[:], RMAGIC)
            nc.vector.tensor_scalar(
                out=q2[:], in0=q2[:], scalar1=RMAGIC, scalar2=None,
                op0=OP.subtract,
            )
            par = workp.tile([128, 128], F32, tag="par")
            nc.vector.scalar_tensor_tensor(
                out=par[:], in0=q2[:], scalar=-2.0, in1=fp[:],
                op0=OP.mult, op1=OP.add,
            )
            # y tap arg bases: even rows j' arg = (sY - 4 par) - j',
            # odd rows arg = (sY - 4 + 4 par) - j'
            argE = workp.tile([128, 128], F32, tag="argE")
            nc.vector.scalar_tensor_tensor(
                out=argE[:], in0=par[:], scalar=-4.0, in1=sY[:],
                op0=OP.mult, op1=OP.add,
            )
            argO = workp.tile([128, 128], F32, tag="argO")
            nc.vector.scalar_tensor_tensor(
                out=argO[:], in0=par[:], scalar=4.0, in1=sY[:],
                op0=OP.mult, op1=OP.add,
            )
            nc.vector.tensor_scalar_add(argO[:], argO[:], -4.0)

            # tap weight args: x: xbase - a (8); yE: argE - j' (4); yO (4)
            NXC = 128 * 8
            ut = workp.tile([128, NXC + 128 * 8], F32, tag="ut")
            utx3 = ut[:, 0:NXC].rearrange("p (c a) -> p c a", a=8)
            nc.vector.tensor_tensor(
                out=utx3,
                in0=xbase[:].unsqueeze(2).broadcast_to([128, 128, 8]),
                in1=offs8[:].unsqueeze(1).broadcast_to([128, 128, 8]),
                op=OP.add,
            )
            utyE = ut[:, NXC:NXC + 512].rearrange("p (c j) -> p c j", j=4)
            nc.vector.tensor_tensor(
                out=utyE,
                in0=argE[:].unsqueeze(2).broadcast_to([128, 128, 4]),
                in1=ylat4[:].unsqueeze(1).broadcast_to([128, 128, 4]),
                op=OP.add,
            )
            utyO = ut[:, NXC + 512:].rearrange("p (c j) -> p c j", j=4)
            nc.vector.tensor_tensor(
                out=utyO,
                in0=argO[:].unsqueeze(2).broadcast_to([128, 128, 4]),
                in1=ylat4[:].unsqueeze(1).broadcast_to([128, 128, 4]),
                op=OP.add,
            )
            # t = max(0, 1 - (U/3)^2), Horner in t
            tsq = workp.tile([128, NXC + 128 * 8], F32, tag="tsq")
            nc.vector.tensor_mul(tsq[:], ut[:], ut[:])
            nc.vector.tensor_scalar(
                out=tsq[:], in0=tsq[:], scalar1=float(-1.0 / 9.0), scalar2=1.0,
                op0=OP.mult, op1=OP.add,
            )
            nc.vector.tensor_scalar_max(tsq[:], tsq[:], 0.0)
            acc = workp.tile([128, NXC + 128 * 8], F32, tag="acc")
            nc.vector.tensor_scalar(
                out=acc[:], in0=tsq[:], scalar1=float(CH[DEG]),
                scalar2=float(CH[DEG - 1]), op0=OP.mult, op1=OP.add,
            )
            for d in range(DEG - 2, -1, -1):
                nc.vector.tensor_mul(acc[:], acc[:], tsq[:])
                nc.vector.tensor_scalar_add(acc[:], acc[:], float(CH[d]))
            # acc = [x taps (c,8) | yE (c,4) | yO (c,4)]

            # outer-product weights wqv[par][p, (c, q8, v4)] in bf16
            wqvE = workp.tile([128, 128 * 32], BF16, tag="wqvE")
            wqvO = workp.tile([128, 128 * 32], BF16, tag="wqvO")
            for wqv, yoff in ((wqvE, NXC), (wqvO, NXC + 512)):
                ov = bass.AP(wqv[:].tensor, wqv[:].offset,
                             [wqv[:].ap[0], [32, 128], [4, 8], [1, 4]])
                wxv = bass.AP(acc[:].tensor, acc[:].offset,
                              [acc[:].ap[0], [8, 128], [1, 8], [0, 4]])
                wyv = bass.AP(acc[:].tensor, acc[:].offset + yoff,
                              [acc[:].ap[0], [4, 128], [0, 8], [1, 4]])
                nc.vector.tensor_tensor(out=ov, in0=wxv, in1=wyv, op=OP.mult)

            # ======== fold-layout idx math -> int16 via D4 ========
            # kf[d] is host-shuffled so a plain [128,128] load gives
            # fold[P = pd'*16+q, j = c'*8+pd] = k[d, point(P, j)]
            kfx = workp.tile([128, 128], F32, tag="kfx")
            kfy = workp.tile([128, 128], F32, tag="kfy")
            nc.sync.dma_start(out=kfx[:], in_=kf_in[0])
            nc.sync.dma_start(out=kfy[:], in_=kf_in[1])
            fgx = workp.tile([128, 256], F32, tag="fgx")
            nc.vector.tensor_scalar_mul(fgx[:, 0:128], kfx[:], float(G / TWO_PI))
            nc.vector.tensor_scalar_mul(fgx[:, 128:256], kfy[:], float(G / TWO_PI))
            fmsk = workp.tile([128, 256], F32, tag="fmsk")
            nc.vector.tensor_scalar(
                out=fmsk[:], in0=fgx[:], scalar1=0.0, scalar2=None, op0=OP.is_lt
            )
            nc.vector.scalar_tensor_tensor(
                out=fgx[:], in0=fmsk[:], scalar=float(G), in1=fgx[:],
                op0=OP.mult, op1=OP.add,
            )
            ffl = workp.tile([128, 256], F32, tag="ffl")
            nc.vector.tensor_scalar(
                out=ffl[:], in0=fgx[:], scalar1=-3.0 + FEPS, scalar2=RMAGIC,
                op0=OP.add, op1=OP.add,
            )
            nc.vector.tensor_scalar(
                out=ffl[:], in0=ffl[:], scalar1=RMAGIC, scalar2=None,
                op0=OP.subtract,
            )
            # fu = floor((ffl_x+3)/4)
            fu = workp.tile([128, 128], F32, tag="fu")
            nc.vector.tensor_scalar(
                out=fu[:], in0=ffl[:, 0:128], scalar1=0.25,
                scalar2=0.75 + FEPS, op0=OP.mult, op1=OP.add,
            )
            nc.vector.tensor_scalar_add(fu[:], fu[:], RMAGIC)
            nc.vector.tensor_scalar(
                out=fu[:], in0=fu[:], scalar1=RMAGIC, scalar2=None,
                op0=OP.subtract,
            )
            # ffp = Q0
            ffp = workp.tile([128, 128], F32, tag="ffp")
            nc.vector.tensor_scalar(
                out=ffp[:], in0=ffl[:, 128:256], scalar1=0.25,
                scalar2=1.25 + FEPS, op0=OP.mult, op1=OP.add,
            )
            nc.vector.tensor_scalar_add(ffp[:], ffp[:], RMAGIC)
            nc.vector.tensor_scalar(
                out=ffp[:], in0=ffp[:], scalar1=RMAGIC, scalar2=None,
                op0=OP.subtract,
            )
            fq2 = workp.tile([128, 128], F32, tag="fq2")
            nc.vector.tensor_scalar(
                out=fq2[:], in0=ffp[:], scalar1=0.5, scalar2=-0.25,
                op0=OP.mult, op1=OP.add,
            )
            nc.vector.tensor_scalar_add(fq2[:], fq2[:], RMAGIC)
            nc.vector.tensor_scalar(
                out=fq2[:], in0=fq2[:], scalar1=RMAGIC, scalar2=None,
                op0=OP.subtract,
            )
            fpar = workp.tile([128, 128], F32, tag="fpar")
            nc.vector.scalar_tensor_tensor(
                out=fpar[:], in0=fq2[:], scalar=-2.0, in1=ffp[:],
                op0=OP.mult, op1=OP.add,
            )
            fe = workp.tile([128, 128], F32, tag="fe")
            nc.vector.tensor_add(fe[:], fq2[:], fpar[:])
            fme = workp.tile([128, 128], F32, tag="fme")
            nc.vector.tensor_scalar(
                out=fme[:], in0=fe[:], scalar1=63.5, scalar2=None, op0=OP.is_gt
            )
            nc.vector.scalar_tensor_tensor(
                out=fe[:], in0=fme[:], scalar=-64.0, in1=fe[:],
                op0=OP.mult, op1=OP.add,
            )
            fmo = workp.tile([128, 128], F32, tag="fmo")
            nc.vector.tensor_scalar(
                out=fmo[:], in0=fq2[:], scalar1=63.5, scalar2=None, op0=OP.is_gt
            )
            fo = workp.tile([128, 128], F32, tag="fo")
            nc.vector.scalar_tensor_tensor(
                out=fo[:], in0=fmo[:], scalar=-64.0, in1=fq2[:],
                op0=OP.mult, op1=OP.add,
            )
            fiE = workp.tile([128, 128], F32, tag="fiE")
            nc.vector.scalar_tensor_tensor(
                out=fiE[:], in0=fe[:], scalar=129.0, in1=fu[:],
                op0=OP.mult, op1=OP.add,
            )
            fiO = workp.tile([128, 128], F32, tag="fiO")
            nc.vector.scalar_tensor_tensor(
                out=fiO[:], in0=fo[:], scalar=129.0, in1=fu[:],
                op0=OP.mult, op1=OP.add,
            )
            fi16 = workp.tile([128, 256], I16, tag="fi16")
            nc.vector.tensor_copy(out=fi16[:, 0:128], in_=fiE[:])
            nc.vector.tensor_copy(out=fi16[:, 128:256], in_=fiO[:])
            d4w = nc.sync.dma_start(out=D4[:], in_=fi16[:])
            # replicate to idx16 [128, 2048]: [even 1024 | odd 1024];
            # group r partition q, col pd'*128+j <- D4[pd'*16+q, j]
            idx16 = workp.tile([128, 2048], I16, tag="idx16")
            d4r = []
            for r in range(8):
                for half in range(2):
                    src = bass.AP(
                        D4, half * 128,
                        [[256, 16], [4096, 8], [1, 128]],
                    )
                    d4r.append(nc.sync.dma_start(
                        out=idx16[r * 16:(r + 1) * 16,
                                  half * 1024:(half + 1) * 1024],
                        in_=src,
                    ))
            for rd in d4r:
                tile.add_dep_helper(rd.ins, d4w.ins, reason="D4 RAW")

            # ---------------- x image tiles ----------------
            xts = []
            for xt in range(2):
                xt_t = workp.tile([128, 2 * IM], F32, tag=f"xt{xt}")
                nc.sync.dma_start(
                    out=xt_t[:],
                    in_=x_in[:, xt * 128:(xt + 1) * 128, :]
                    .rearrange("ri x y -> x ri y"),
                )
                xts.append(xt_t)

            # stg0: partition p = table row (p<64: even row p, else odd
            # row p-64); el (q, r4, cri)
            stg0 = stgp.tile([128, G * W2], BF16, tag="stg0")

            for c in range(NC):
                # ---- coil multiply (bf16 out for PE) ----
                mt = []
                for xt in range(2):
                    ct = coilp.tile([128, 2 * IM], F32, tag="ct")
                    nc.sync.dma_start(
                        out=ct[:],
                        in_=c_in[c, :, xt * 128:(xt + 1) * 128, :]
                        .rearrange("ri x y -> x ri y"),
                    )
                    xt_t = xts[xt]
                    m = mp.tile([128, 2 * IM], BF16, tag="m")
                    xr, xi = xt_t[:, 0:IM], xt_t[:, IM:2 * IM]
                    cr, ci = ct[:, 0:IM], ct[:, IM:2 * IM]
                    mr, mi = m[:, 0:IM], m[:, IM:2 * IM]
                    t1 = mp.tile([128, IM], F32, tag="cm1")
                    t2 = mp.tile([128, IM], F32, tag="cm2")
                    nc.vector.tensor_mul(t1[:], xr, cr)
                    nc.vector.tensor_mul(t2[:], xi, ci)
                    nc.vector.tensor_sub(mr, t1[:], t2[:])
                    nc.vector.tensor_mul(t1[:], xr, ci)
                    nc.vector.tensor_mul(t2[:], xi, cr)
                    nc.vector.tensor_add(mi, t1[:], t2[:])
                    mt.append(m)
                # ---- stage 1: BT[y, u] per (ri, Yt) ----
                bt = {}
                for yt in range(2):
                    pr = ps1.tile([128, G], F32, tag="psa")
                    pi = ps1.tile([128, G], F32, tag="psa")
                    for xt in range(2):
                        mrb = mt[xt][:, yt * 128:yt * 128 + 128]
                        mib = mt[xt][:, IM + yt * 128:IM + yt * 128 + 128]
                        st = xt == 0
                        sp = xt == 1
                        nc.tensor.matmul(pr[:], mrb, artT[xt][:], start=st, stop=False)
                        nc.tensor.matmul(pi[:], mrb, aitT[xt][:], start=st, stop=False)
                        nc.tensor.matmul(pr[:], mib, aitnT[xt][:], start=False, stop=sp)
                        nc.tensor.matmul(pi[:], mib, artT[xt][:], start=False, stop=sp)
                    btr = btp.tile([128, G], BF16, tag="bt")
                    bti = btp.tile([128, G], BF16, tag="bt")
                    nc.scalar.copy(out=btr[:], in_=pr[:])
                    nc.scalar.copy(out=bti[:], in_=pi[:])
                    bt[(0, yt)] = btr
                    bt[(1, yt)] = bti
                # ---- stage 2: G[v, u], v = 4*V(p) + r2 via permuted A
                # column slices; drain into staging ----
                for r2 in range(4):
                    stg3 = stg0[:].rearrange("p (q w) -> p q w", w=W2)
                    gr = ps2.tile([128, G], F32, tag="psb")
                    gi = ps2.tile([128, G], F32, tag="psb")
                    for yt in range(2):
                        av = artP[yt][:, r2 * 128:(r2 + 1) * 128]
                        aiv = aitP[yt][:, r2 * 128:(r2 + 1) * 128]
                        ainv = aitnP[yt][:, r2 * 128:(r2 + 1) * 128]
                        btr = bt[(0, yt)]
                        bti = bt[(1, yt)]
                        st = yt == 0
                        sp = yt == 1
                        nc.tensor.matmul(gr[:], av, btr[:], start=st, stop=False)
                        nc.tensor.matmul(gi[:], aiv, btr[:], start=st, stop=False)
                        nc.tensor.matmul(gr[:], ainv, bti[:], start=False, stop=sp)
                        nc.tensor.matmul(gi[:], av, bti[:], start=False, stop=sp)
                    c2 = r2 * CELL + 2 * c
                    # split strided drains across Scalar and Vector engines
                    if c % 2 == 0:
                        nc.scalar.copy(out=stg3[:, :, c2:c2 + 1], in_=gr[:].unsqueeze(2))
                        nc.vector.tensor_copy(out=stg3[:, :, c2 + 1:c2 + 2], in_=gi[:].unsqueeze(2))
                    else:
                        nc.vector.tensor_copy(out=stg3[:, :, c2:c2 + 1], in_=gr[:].unsqueeze(2))
                        nc.scalar.copy(out=stg3[:, :, c2 + 1:c2 + 2], in_=gi[:].unsqueeze(2))

            # ---- table stores: contiguous partition blocks ----
            t_stores = []
            for T_, plo in ((TE, 0), (TO, 64)):
                t_stores.append(nc.sync.dma_start(
                    out=T_[0:64, 2 * W2:514 * W2],
                    in_=stg0[plo:plo + 64, 0:512 * W2],
                ))
                t_stores.append(nc.sync.dma_start(
                    out=T_[0:64, 514 * W2:516 * W2],
                    in_=stg0[plo:plo + 64, 0:2 * W2],
                ))
                t_stores.append(nc.sync.dma_start(
                    out=T_[0:64, 0:2 * W2],
                    in_=stg0[plo:plo + 64, 510 * W2:512 * W2],
                ))

            # ======== gather + combine ========
            # 64*129 = 8256 units of 256 el; idx max 8254, fetch 2 units
            teap = bass.AP(TE, 0, [[256, 8255], [1, 512]])
            toap = bass.AP(TO, 0, [[256, 8255], [1, 512]])
            ECOLS = 8 * 512
            for t in range(NTILE):
                patches = []
                for tab, ioff in ((teap, 0), (toap, 1024)):
                    patch = patchp.tile([128, ECOLS], BF16, tag="patch")
                    gi_ = nc.gpsimd.dma_gather(
                        patch[:].rearrange("p (g e) -> p g e", e=512),
                        tab,
                        idx16[:, ioff + t * 64:ioff + (t + 1) * 64],
                        1024,
                        1024,
                        512,
                        elem_step=256,
                    )
                    for si in t_stores:
                        tile.add_dep_helper(gi_.ins, si.ins, reason="T RAW")
                    patches.append(patch)
                # stage A: multiply by broadcast outer-product weights
                for patch, wqv in ((patches[0], wqvE), (patches[1], wqvO)):
                    pv = bass.AP(patch[:].tensor, patch[:].offset,
                                 [patch[:].ap[0], [512, 8], [64, 8],
                                  [16, 4], [1, 16]])
                    wv = bass.AP(wqv[:].tensor, wqv[:].offset + t * 8 * 32,
                                 [wqv[:].ap[0], [32, 8], [4, 8],
                                  [1, 4], [0, 16]])
                    nc.vector.tensor_tensor(out=pv, in0=pv, in1=wv, op=OP.mult)
                    # q tree: 8 -> 4 -> 2 -> 1 (in place, bf16)
                    for lv, (qn, qs) in enumerate(((4, 4), (2, 2), (1, 1))):
                        o = bass.AP(patch[:].tensor, patch[:].offset,
                                    [patch[:].ap[0], [512, 8], [64, qn],
                                     [1, 64]])
                        i1 = bass.AP(patch[:].tensor,
                                     patch[:].offset + qs * 64,
                                     [patch[:].ap[0], [512, 8], [64, qn],
                                      [1, 64]])
                        nc.vector.tensor_tensor(out=o, in0=o, in1=i1, op=OP.add)
                # v sum (f32 out): v0+v2, +v1, +v3 per parity, then merge
                pe, po = patches
                v2 = resp.tile([128, 8 * 32], F32, tag="v2")
                for src, half in ((pe, 0), (po, 1)):
                    o = bass.AP(v2[:].tensor, v2[:].offset + half * 16,
                                [v2[:].ap[0], [32, 8], [1, 16]])
                    ia = bass.AP(src[:].tensor, src[:].offset,
                                 [src[:].ap[0], [512, 8], [1, 16]])
                    ib = bass.AP(src[:].tensor, src[:].offset + 32,
                                 [src[:].ap[0], [512, 8], [1, 16]])
                    nc.vector.tensor_tensor(out=o, in0=ia, in1=ib, op=OP.add)
                    for voff in (16, 48):
                        iv = bass.AP(src[:].tensor, src[:].offset + voff,
                                     [src[:].ap[0], [512, 8], [1, 16]])
                        nc.vector.tensor_tensor(out=o, in0=o, in1=iv, op=OP.add)
                rt = resp.tile([128, 128], F32, tag="rt")
                rv = rt[:].rearrange("p (g cr) -> p g cr", cr=16)
                ea = bass.AP(v2[:].tensor, v2[:].offset,
                             [v2[:].ap[0], [32, 8], [1, 16]])
                ob = bass.AP(v2[:].tensor, v2[:].offset + 16,
                             [v2[:].ap[0], [32, 8], [1, 16]])
                nc.vector.tensor_tensor(out=rv, in0=ea, in1=ob, op=OP.add)
                ts_ = slice(t * 128, (t + 1) * 128)
                nc.vector.tensor_mul(rt[:], rt[:], wsq[:, ts_])
                nc.sync.dma_start(out=y_out[:, ts_], in_=rt[:])

            if debug:
                dbg_outs = {
                    "kgo": kg, "acco": acc, "fi16o": fi16, "idx16o": idx16,
                    "flo": fl, "rro": rr, "fpo": fp, "wqvEo": wqvE,
                    "wqvOo": wqvO, "stg0o": stg0,
                }
                for nm, t_ in dbg_outs.items():
                    o = nc.dram_tensor(nm, list(t_[:].shape), t_[:].dtype,
                                       kind="ExternalOutput")
                    nc.sync.dma_start(out=o[:], in_=t_[:])
                for nm, T_ in (("teo", TE), ("too", TO)):
                    o = nc.dram_tensor(nm, [64, TROW], BF16,
                                       kind="ExternalOutput")
                    di = nc.sync.dma_start(out=o[:], in_=T_[:])
                    for si in t_stores:
                        tile.add_dep_helper(di.ins, si.ins, reason="T dump")

    nc.compile()
    return nc


_NC_CACHE = None


def _get_nc():
    global _NC_CACHE
    if _NC_CACHE is None:
        _NC_CACHE = build_bass()
    return _NC_CACHE


# ---------------------------------------------------------------- host glue
def _shuffle_w(w_t):
    # w[c, ri, K] -> [p, (t, g, c, ri)] with K = t*1024 + g*128 + p
    v = w_t.reshape(NC, 2, NTILE, GRP, 128)
    return np.ascontiguousarray(v.transpose(4, 2, 3, 0, 1).reshape(128, NTILE * 128))


def _unshuffle_y(yr):
    # [p, (t, g, c, ri)] -> y[c, ri, K]
    v = yr.reshape(128, NTILE, GRP, NC, 2)
    return np.ascontiguousarray(v.transpose(3, 4, 1, 2, 0).reshape(NC, 2, K))


def _fold_k(k_t):
    # fold[P = pd'*16+q, j = c'*8+pd] = k[d, (pd'*16+c')*128 + pd*16 + q]
    # K index = c*128 + p = (pdp*16 + cp)*128 + pd*16 + q
    v = np.asarray(k_t, dtype=np.float32).reshape(2, 8, 16, 8, 16)
    # dims: [d, pdp, cp, pd, q] -> fold[d, (pdp, q), (cp, pd)]
    f = v.transpose(0, 1, 4, 2, 3).reshape(2, 128, 128)
    return np.ascontiguousarray(f)


def make_in_maps(x, k, coil_sensitivities, w):
    in_maps = []
    coil0 = np.ascontiguousarray(coil_sensitivities[0], dtype=np.float32)
    for t in range(NT):
        kt = np.ascontiguousarray(k[t], dtype=np.float32)
        in_maps.append({
            "x": np.ascontiguousarray(x[t], dtype=np.float32),
            "kk": kt,
            "kf": _fold_k(kt),
            "coil": coil0,
            "wr": _shuffle_w(np.asarray(w[t], dtype=np.float32)),
            "art": _ART, "ait": _AIT, "aitn": _AITN,
        })
    return in_maps


def run(x, k, coil_sensitivities, w, trace=False, **spmd_kwargs):
    nc = _get_nc()
    in_maps = make_in_maps(x, k, coil_sensitivities, w)
    r = run_bass_kernel_spmd(nc, in_maps, list(range(NT)), trace=trace, **spmd_kwargs)
    y = np.stack([_unshuffle_y(r.results[t]["yr"]) for t in range(NT)], axis=0)
    return y.astype(np.float32), r


def kernel(x, k, coil_sensitivities, w):
    y, _ = run(x, k, coil_sensitivities, w, trace=False)
    return y
